# revision 4
# baseline (speedup 1.0000x reference)
"""AttnBlock (GroupNorm + single-head LxL attention + residual) on 8 trn2 cores.

Data-parallel over batch: core b handles sample b (full 2048x2048 attention).
All big matmuls run as fp8 e4m3 with MatmulPerfMode.DoubleRow (256-row
contraction per instruction, two 128-row slabs).

Host-side prep (f32, in make_in_maps):
- GroupNorm statistics: per-channel s = gn_w * rstd(group), t = gn_b -
  mean(group) * s ship as a tiny [128, 2*CT] tensor; the device only applies
  h = s*x + t (chunked, overlapped with the x DMA halves).
- Weight composition: scores need z[i,j] = h_i^T (Wq^T Wk) h_j + (Wk^T bq).
  h_j + f(i) + const, and f(i)/const cancel in softmax over j, so Q/K convs
  collapse into one U = (Wq^T Wk) h conv plus a per-j exp bias
  r2[j] = (Wk^T bq).h_j. The value path collapses too:
  Wo(Wv h + bv) = (Wo Wv) h + Wo bv, with Wo bv + bo folded into the
  host-transposed residual.

Device layout (per core):
  x             : (C, L) fp8, 8 half tiles [128, 1024] split over both
                  hwdge queues; GN apply consumes them as they land
                  (residual precision comes from the separate bf16 x^T)
  h8, u8        : fp8 pair tiles [128, 2, 2048]; slab s = channels
                  cp*256 + s*128 .. +128 (DoubleRow contraction pairs)
  Wu8, Wvo8     : fp8 pair tiles [128, 2, 512], host-scaled by 16 (e4m3
                  range); the 1/16 is folded into the psum evacuation
  S^T = U^T H   : [j, i] psum tiles; ACT exp with bias (-3*ln2 + scale*r2[j])
                  writes fp8 P~^T pair tiles [128, 2, 512] (pairs over jt)
  W~^T          : fp8 pair tiles [128, 2, 512] (pairs over jt)
  out^T[i, o]   : psum accum over 8 jt pairs; rowsums accumulate as [1, 512]
                  fp8-ones DoubleRow matmuls, transposed to per-partition
                  [128, 4] via K=1 matmuls; the block tail splits the
                  normalize+residual evac across DVE/ACT/GpSimd so the next
                  block's psum banks free early.
  The output is written as (L, C) and transposed back on the host.
"""

import numpy as np
import ml_dtypes

C = 512
L = 2048
G = 32
GS = C // G          # 16 channels per group
EPS = 1e-6
CT = C // 128        # 4 channel tiles
CP = 2               # channel slab pairs
JT = L // 128        # 16 j tiles
JP = JT // 2         # 8 j tile pairs
NB = 512             # matmul moving free dim / chunk size
LB = L // NB         # 4 i-blocks
NCORES = 8
WSCALE = 16.0        # host weight prescale (folded out at psum evac)
PBIAS = -3.0 * float(np.log(2.0))  # exp bias: p-scale 1/8, cancels in norm

F8NP = ml_dtypes.float8_e4m3
BF16NP = ml_dtypes.bfloat16

_CACHE = {}


def _build():
    import concourse.bacc as bacc
    import concourse.tile as tile
    from concourse import mybir
    from concourse.alu_op_type import AluOpType
    from contextlib import ExitStack

    F32 = mybir.dt.float32
    BF16 = mybir.dt.bfloat16
    F8 = mybir.dt.float8e4
    AF = mybir.ActivationFunctionType
    DR = mybir.MatmulPerfMode.DoubleRow

    nc = bacc.Bacc("TRN2", target_bir_lowering=False, debug=False, num_devices=NCORES)

    _ctr = [0]

    def nm(base):
        _ctr[0] += 1
        return f"{base}_{_ctr[0]}"

    x_d = nc.declare_dram_parameter("x8", [C // 2, 2, L], F8, isOutput=False)
    xt_d = nc.declare_dram_parameter("xt", [L, C], F32, isOutput=False)
    wu_d = nc.declare_dram_parameter("wu8", [C // 2, 2, C], F8, isOutput=False)
    wvo_d = nc.declare_dram_parameter("wvo8", [C // 2, 2, C], F8, isOutput=False)
    eb_d = nc.declare_dram_parameter("ebias", [128, JT], F32, isOutput=False)
    one_d = nc.declare_dram_parameter("one8", [128, 2, 16], F8, isOutput=False)
    yt_d = nc.declare_dram_parameter("yt", [L, C], F32, isOutput=True)

    scale = float(1.0 / np.sqrt(C))
    winv = float(1.0 / WSCALE)
    HB = L // 2          # x arrives in half tiles of 1024 columns

    with tile.TileContext(nc) as tc, ExitStack() as ctx:
        consts = ctx.enter_context(tc.tile_pool(name="consts", bufs=1))
        small = ctx.enter_context(tc.tile_pool(name="small", bufs=4))
        x_p = ctx.enter_context(tc.tile_pool(name="xp", bufs=1))
        h_p = ctx.enter_context(tc.tile_pool(name="hp", bufs=1))
        u_p = ctx.enter_context(tc.tile_pool(name="up", bufs=1))
        w_p = ctx.enter_context(tc.tile_pool(name="wp", bufs=1))
        wt_p = ctx.enter_context(tc.tile_pool(name="wtp", bufs=1))
        p_p = ctx.enter_context(tc.tile_pool(name="pp", bufs=4))
        io_p = ctx.enter_context(tc.tile_pool(name="io", bufs=4))
        xt_p = ctx.enter_context(tc.tile_pool(name="xtp", bufs=5))
        # 8 psum banks: 4 out-accumulators + 3 rotating scores bufs (the exp
        # at ~640ns is slower than the 2 score matmuls it gates) + 1 small
        ps_mm = ctx.enter_context(tc.tile_pool(name="psmm", bufs=4, space="PSUM"))
        ps_sc = ctx.enter_context(tc.tile_pool(name="pssc", bufs=3, space="PSUM"))
        ps_rs = ctx.enter_context(tc.tile_pool(name="psrs", bufs=1, space="PSUM"))

        # warmup operand: a DVE-memset const tile, available immediately
        wc = consts.tile([128, 128], F32, name=nm("wc"), tag="wc")
        nc.vector.memset(wc[:], 0.25)

        # ---- param loads, interleaved across the two hwdge queues so the
        # GN-apply/U-conv pipeline can start before x fully lands ----
        # sync  : x0a, wu, x2a, x0b, x2b, ones, wr   (+ xt/yt later)
        # scalar: cvec, x1a, x3a, x1b, x3b, wvo
        h8 = [h_p.tile([128, 2, L], F8, name=nm("h"), tag=f"h{cp}")
              for cp in range(CP)]

        def ldx(eng, cp, half):
            eng.dma_start(out=h8[cp][:, :, half * HB:(half + 1) * HB],
                          in_=x_d[cp * 128:(cp + 1) * 128, :,
                                  half * HB:(half + 1) * HB])

        eb_sb = consts.tile([128, JT], F32, name=nm("eb"), tag="eb")
        nc.scalar.dma_start(out=eb_sb[:], in_=eb_d[:, :])
        ldx(nc.sync, 0, 0)
        ldx(nc.scalar, 1, 0)
        w_sb = {"u": [], "vo": []}
        wu_tiles = [w_p.tile([128, 2, C], F8, name=nm("w"), tag=f"wu{cp}")
                    for cp in range(CP)]
        w_sb["u"] = wu_tiles
        nc.sync.dma_start(out=wu_tiles[0][:], in_=wu_d[0:128, :, :])
        nc.scalar.dma_start(out=wu_tiles[1][:], in_=wu_d[128:256, :, :])
        ldx(nc.sync, 0, 1)
        ldx(nc.scalar, 1, 1)
        for cp in range(CP):
            w = w_p.tile([128, 2, C], F8, name=nm("w"), tag=f"wvo{cp}")
            nc.scalar.dma_start(out=w[:], in_=wvo_d[cp * 128:(cp + 1) * 128, :, :])
            w_sb["vo"].append(w)
        ones_t = consts.tile([128, 2, 16], F8, name=nm("ones"), tag="ones")
        nc.sync.dma_start(out=ones_t[:], in_=one_d[:, :, :])
        onesf = consts.tile([1, 1], F32, name=nm("onesf"), tag="onesf")
        nc.vector.memset(onesf[:], 1.0)
        # warm-up matmuls: keep the PE clock ramped while x streams in
        for i in range(8):
            wps = ps_mm.tile([128, 128], F32, name=nm("warm"), tag="mm")
            nc.tensor.matmul(wps[:], wc[:], wc[:], start=True, stop=True)

        # ---- U' = (S Wq^T Wk S) x conv: lc outer, pipelined behind x DMA ----
        u8 = [u_p.tile([128, 2, L], F8, name=nm("u"), tag=f"u{cp}")
              for cp in range(CP)]
        for lc in range(L // NB):
            for co in range(CT):
                pss = ps_mm.tile([128, NB], F32, name=nm("mm"), tag="mm")
                for cp in range(CP):
                    nc.tensor.matmul(
                        pss[:],
                        w_sb["u"][cp][:, :, co * 128:(co + 1) * 128],
                        h8[cp][:, :, lc * NB:(lc + 1) * NB],
                        start=(cp == 0), stop=(cp == CP - 1), perf_mode=DR)
                o = u8[co // 2][:, co % 2, lc * NB:(lc + 1) * NB]
                if co % 2 == 0:
                    nc.vector.tensor_scalar_mul(out=o, in0=pss[:],
                                                scalar1=winv)
                else:
                    nc.scalar.activation(out=o, in_=pss[:], func=AF.Identity,
                                         scale=winv)

        # ---- W~^T = ((Wo Wv) h)^T: (L, C)-oriented fp8 pair tiles over jt ----
        wt8 = [wt_p.tile([128, 2, C], F8, name=nm("wt"), tag=f"wt{jp}")
               for jp in range(JP)]
        for jt in range(JT):
            pw = ps_mm.tile([128, C], F32, name=nm("mm"), tag="mm")
            for cp in range(CP):
                nc.tensor.matmul(
                    pw[:],
                    h8[cp][:, :, jt * 128:(jt + 1) * 128],
                    w_sb["vo"][cp][:, :, :],
                    start=(cp == 0), stop=(cp == CP - 1), perf_mode=DR)
            o = wt8[jt // 2][:, jt % 2, :]
            if jt % 2 == 0:
                nc.vector.tensor_scalar_mul(out=o, in0=pw[:], scalar1=winv)
            else:
                nc.scalar.activation(out=o, in_=pw[:], func=AF.Identity,
                                     scale=winv)

        # ---- attention: blocks of 512 i columns ----
        for ib in range(LB):
            rsps = ps_rs.tile([1, NB], F32, name=nm("rs"), tag="rs")
            ops = [ps_mm.tile([128, C], F32, name=nm("mm"), tag="mm")
                   for _ in range(4)]
            xt_sbs = []
            for s in range(4):
                row = ib * NB + s * 128
                xt_sb = xt_p.tile([128, C], F32, name=nm("xt"), tag="xt")
                nc.sync.dma_start(out=xt_sb[:], in_=xt_d[row:row + 128, :])
                xt_sbs.append(xt_sb)
            # software-pipelined: the rowsum/out matmuls of jp-1 are emitted
            # AFTER the scores of jp, so the in-order PE queue never waits on
            # the two serial ACT exps (1.3us) that pt depends on
            def rsout(pt, jp):
                # row sums first: the block-tail normalize chain hangs off
                # this, so it should finish before the last out matmuls
                nc.tensor.matmul(rsps[:], ones_t[:, :, 0:1], pt[:, :, :],
                                 start=(jp == 0), stop=(jp == JP - 1),
                                 perf_mode=DR)
                for s in range(4):
                    nc.tensor.matmul(ops[s][:],
                                     pt[:, :, s * 128:(s + 1) * 128],
                                     wt8[jp][:, :, :],
                                     start=(jp == 0), stop=(jp == JP - 1),
                                     perf_mode=DR)

            prev = None
            for jp in range(JP):
                pt = p_p.tile([128, 2, NB], F8, name=nm("p"), tag="p")
                for half in range(2):
                    jt = 2 * jp + half
                    sps = ps_sc.tile([128, NB], F32, name=nm("s"), tag="sc")
                    for cp in range(CP):
                        nc.tensor.matmul(
                            sps[:],
                            u8[cp][:, :, jt * 128:(jt + 1) * 128],
                            h8[cp][:, :, ib * NB:(ib + 1) * NB],
                            start=(cp == 0), stop=(cp == CP - 1),
                            perf_mode=DR)
                    nc.scalar.activation(out=pt[:, half, :], in_=sps[:],
                                         func=AF.Exp, scale=scale,
                                         bias=eb_sb[:, jt:jt + 1])
                if prev is not None:
                    rsout(*prev)
                prev = (pt, jp)
            rsout(*prev)
            # rowsum -> per-partition reciprocal via K=1 transpose matmuls.
            # The chain gates the ops-psum release (next block's out matmuls
            # reuse the banks), so it is split across engines: DVE does the
            # fused (psum*rec)+xt for s0/s2, ACT mul + DVE/gpsimd add free
            # the s1/s3 banks early.
            rssb = small.tile([1, NB], F32, name=nm("rssb"), tag="rssb")
            nc.vector.tensor_copy(out=rssb[:], in_=rsps[:])
            rec4 = small.tile([128, 4], F32, name=nm("rec4"), tag="rec4")
            trp = ps_rs.tile([128, 4], F32, name=nm("tr"), tag="rs")
            for s in range(4):
                nc.tensor.matmul(trp[:, s:s + 1],
                                 rssb[0:1, s * 128:(s + 1) * 128],
                                 onesf[:],
                                 start=True, stop=True)
            nc.vector.reciprocal(out=rec4[:], in_=trp[:])
            for s in range(4):
                rec = rec4[:, s:s + 1]
                row = ib * NB + s * 128
                yt_sb = io_p.tile([128, C], F32, name=nm("yt"), tag="yt")
                if s % 2 == 0:
                    nc.vector.scalar_tensor_tensor(out=yt_sb[:], in0=ops[s][:],
                                                   scalar=rec,
                                                   in1=xt_sbs[s][:],
                                                   op0=AluOpType.mult,
                                                   op1=AluOpType.add)
                else:
                    o1 = io_p.tile([128, C], F32, name=nm("o1"), tag="o1")
                    nc.scalar.activation(out=o1[:], in_=ops[s][:],
                                         func=AF.Copy, scale=rec)
                    eng = nc.gpsimd if ib < LB - 1 else nc.vector
                    eng.tensor_add(out=yt_sb[:], in0=o1[:], in1=xt_sbs[s][:])
                nc.sync.dma_start(out=yt_d[row:row + 128, :], in_=yt_sb[:])

    nc.compile()
    return nc


def get_nc():
    if "nc" not in _CACHE:
        _CACHE["nc"] = _build()
    return _CACHE["nc"]


def _pair8(wT):
    # (C_in, O) f32 -> fp8 pair layout [C_in//2, 2, O]:
    # [cp*128+p, s, o] = wT[cp*256 + s*128 + p, o]
    O = wT.shape[1]
    return np.ascontiguousarray(
        wT.reshape(2, 2, 128, O).transpose(0, 2, 1, 3)).reshape(
            C // 2, 2, O).astype(F8NP)


def make_in_maps(**inputs):
    x = np.asarray(inputs["x"], np.float32)
    wq = np.asarray(inputs["wq"], np.float32)
    wk = np.asarray(inputs["wk"], np.float32)
    wv = np.asarray(inputs["wv"], np.float32)
    wo = np.asarray(inputs["wo"], np.float32)
    bq = np.asarray(inputs["bq"], np.float32)
    bv = np.asarray(inputs["bv"], np.float32)
    bo = np.asarray(inputs["bo"], np.float32)
    gn_w = np.asarray(inputs["gn_w"], np.float32)
    gn_b = np.asarray(inputs["gn_b"], np.float32)
    # composed weights (f32 on host); the conv contracts over the FIRST
    # host index. GroupNorm h = S x + t folds into every weight:
    #   scores: h^T A h = x^T (S A S) x + (S A^T t + S Wk^T bq).x_j + (terms
    #           constant in j, which softmax cancels)     [A = Wq^T Wk]
    #   values: (Wo Wv) h + Wo bv = (Wo Wv S) x + (Wo Wv t + Wo bv)
    wu = wk.T @ wq                 # = A^T; conv needs W_host with W^T = A
    wvoT = np.ascontiguousarray((wo @ wv).T)  # [c_in, o]
    wrb = wk.T @ bq                # bq scores term, per-j via x
    gvo = wo @ wv                  # value-path matrix [o, c]
    B = x.shape[0]
    xg = x.reshape(B, G, GS * L)
    mean = xg.mean(axis=2)
    var = xg.var(axis=2)
    rstd = 1.0 / np.sqrt(var + EPS)
    sc = gn_w[None, :] * np.repeat(rstd, GS, axis=1)       # [B, C]
    tc = gn_b[None, :] - np.repeat(mean, GS, axis=1) * sc
    shared = {
        "one8": np.ones((128, 2, 16), np.float32).astype(F8NP),
    }
    in_maps = []
    for b in range(B):
        m = dict(shared)
        sb, tb = sc[b], tc[b]
        m["wu8"] = _pair8((sb[:, None] * wu * sb[None, :]) * WSCALE)
        m["wvo8"] = _pair8((sb[:, None] * wvoT) * WSCALE)
        # per-j exp bias: PBIAS + scale * (S A^T t + S Wk^T bq).x_j
        wr = sb * (wu @ tb + wrb)
        r2 = wr @ x[b]                                     # [L]
        m["ebias"] = np.ascontiguousarray(
            (PBIAS + float(1.0 / np.sqrt(C)) * r2).reshape(JT, 128).T)
        res_b = gvo @ tb + wo @ bv + bo
        m["x8"] = np.ascontiguousarray(
            x[b].reshape(2, 2, 128, L).transpose(0, 2, 1, 3)).reshape(
                C // 2, 2, L).astype(F8NP)
        m["xt"] = np.ascontiguousarray(x[b].T + res_b[None, :])
        in_maps.append(m)
    return in_maps


def kernel(**inputs):
    from concourse.bass_utils import run_bass_kernel_spmd

    nc = get_nc()
    in_maps = make_in_maps(**inputs)
    res = run_bass_kernel_spmd(nc, in_maps, core_ids=list(range(NCORES)))
    out = np.stack([res.results[b]["yt"].T for b in range(NCORES)])
    return np.ascontiguousarray(out, dtype=np.float32)
